# revision 8
# baseline (speedup 1.0000x reference)
"""Trainium2 Bass kernel for nn_AttentionAggregate_Weight (gnn_message_passing).

Computes, per node n with K=32 neighbors and D=128 features:
    score[n,k] = tanh(nodes_key[n].v1 + middle_key[n,k].v2 + a_b)
    out[n,:]   = sum_k softmax_k(score)[n,k] * middle_value[n,k,:]
where v1 = W1.T @ a_w and v2 = W2.T @ a_w are folded on the host (the
reference's p1/p2 projections only ever appear dotted with a_w; tanh
outputs lie in [-1,1] so the softmax needs no max subtraction and the
denominator folds into one final per-node scale).

Distribution: pure data parallel over the node axis across 8 NeuronCores.
Nodes are host-padded 20000 -> 20480 so each core gets 2560 = 20 tiles of
exactly 128 nodes; 128-partition DMA destinations run ~2.5x faster than
the 125-partition shapes the node count would naturally give. The big
inputs are staged as bf16 (host cast), halving HBM traffic; rel-err vs
the fp32 reference is ~5e-3, well inside the 2e-2 gate.

Per 128-node tile: middle_key is staged pre-transposed by the host into
tile-major [d, (k, n)] bf16 slabs (the HW xbar transpose path measured
~2.4x slower than plain loads, so the transpose is done once on the
host) and streams in as plain full-rate loads. The K score dot-products
then run on TensorE as 32 tiny matmuls (stationary = contiguous
mkT[:, k, :] slice, moving = v2 column), accumulating straight into a
[128, K] PSUM tile in node-major layout.
ScalarE applies tanh (center-node term enters via the per-partition bias
port) and exp (+fused row-sum); VectorE does the softmax reciprocal and
the weighted value sum as four bf16 multiply-accumulate sub-chains
(2x-packed DVE mode) combined in fp32, and ScalarE applies the final
1/Z scale. The tile loop is software-pipelined two tiles deep so DMA,
PE, ACT and DVE all overlap; outputs return as bf16 and are upcast on
the host.

Self-contained: hardcodes shapes/sharding; no file I/O.
"""

from contextlib import ExitStack

import numpy as np

N, K, D = 20000, 32, 128
N_CORES = 8
NPC = 2560  # padded nodes per core (20 tiles of 128)
NPAD = NPC * N_CORES
P = 128
SKEW = 2
BUFS = 3
NSUB = 4  # value-chain split for fp32 recombine


# ---------------------------------------------------------------------------
# Wait legalization: this walrus build accepts at most ONE semaphore wait per
# instruction; split extras onto same-engine Drain carriers at the BIR level.
# ---------------------------------------------------------------------------
def _legalize_bir_waits(bir_bytes: bytes) -> bytes:
    import orjson

    m = orjson.loads(bir_bytes)
    n = 0
    for f in m.get("functions", []):
        for b in f.get("blocks", []):
            insts = b.get("instructions") or []
            out = []
            changed = False
            for ins in insts:
                si = ins.get("sync_info")
                waits = (si or {}).get("on_wait") or []
                if len(waits) > 1:
                    changed = True
                    for w in waits[:-1]:
                        n += 1
                        out.append(
                            {
                                "debug": ins.get("debug", 0),
                                "engine": ins.get("engine"),
                                "ins": [],
                                "outs": [],
                                "name": f"I-wfix-{n}",
                                "opcode": "Drain",
                                "sync_info": {"on_update": [], "on_wait": [w]},
                            }
                        )
                    si["on_wait"] = [waits[-1]]
                out.append(ins)
            if changed:
                b["instructions"] = out
    return orjson.dumps(m)


_waitfix_installed = False


def _install_waitfix():
    global _waitfix_installed
    if _waitfix_installed:
        return
    import concourse.bass as bass

    orig = bass.Bass.to_json_bytes

    def patched(self):
        return _legalize_bir_waits(orig(self))

    bass.Bass.to_json_bytes = patched
    _waitfix_installed = True


# ---------------------------------------------------------------------------
# Kernel builder (per-core: NPC nodes, P=128 per tile)
# ---------------------------------------------------------------------------
def _build_kernel(repeat=1):
    import concourse.bass as bass
    import concourse.tile as tile
    from concourse import mybir

    f32 = mybir.dt.float32
    bf16 = mybir.dt.bfloat16
    n_tiles = NPC // P
    KSUB = K // NSUB

    nc = bass.Bass()
    # host-pretransposed, tile-major: row block t*D..(t+1)*D is tile t's
    # [d, (k, p)] slab (k-major free so per-k stationary slices are contiguous)
    mk = nc.dram_tensor("mk", (n_tiles * D, P * K), bf16, kind="ExternalInput")
    nk = nc.dram_tensor("nk", (NPC, D), bf16, kind="ExternalInput")
    mv = nc.dram_tensor("mv", (NPC, K, D), bf16, kind="ExternalInput")
    cb_d = nc.dram_tensor("cb", (128, 1 + D), bf16, kind="ExternalInput")
    cf_d = nc.dram_tensor("cf", (128, 1), f32, kind="ExternalInput")
    out = nc.dram_tensor("out", (NPC, D), bf16, kind="ExternalOutput")

    with tile.TileContext(nc) as tc, ExitStack() as ctx:
        singles = ctx.enter_context(tc.tile_pool(name="singles", bufs=1))
        keys = ctx.enter_context(tc.tile_pool(name="keys", bufs=BUFS + SKEW))
        vals = ctx.enter_context(tc.tile_pool(name="vals", bufs=BUFS + SKEW))
        nks = ctx.enter_context(tc.tile_pool(name="nks", bufs=BUFS))
        outs = ctx.enter_context(tc.tile_pool(name="outs", bufs=BUFS))
        smalls = ctx.enter_context(tc.tile_pool(name="smalls", bufs=BUFS + SKEW))
        accs = ctx.enter_context(tc.tile_pool(name="accs", bufs=2))
        psums = ctx.enter_context(tc.tile_pool(name="psums", bufs=2, space="PSUM"))

        cb = singles.tile([128, 1 + D], bf16)
        nc.gpsimd.dma_start(out=cb, in_=cb_d[:])
        cf = singles.tile([128, 1], f32)
        nc.gpsimd.dma_start(out=cf, in_=cf_d[:])
        v2col = cb[:, 0:1]
        v1row = cb[0:P, 1 : 1 + D]
        ab_sb = cf[0:P, 0:1]
        # dummy touches: engines observe the const-DMA semaphores up front
        dum = singles.tile([1, 2], f32)
        nc.vector.tensor_copy(out=dum[:, 0:1], in_=cf[0:1, 0:1])
        nc.scalar.activation(
            out=dum[:, 1:2], in_=cb[0:1, 0:1],
            func=mybir.ActivationFunctionType.Copy,
        )

        def emit_loads(i, t):
            rows = slice(t * P, (t + 1) * P)
            mkT = keys.tile([D, P * K], bf16, tag="mkT", name=f"mkT_{i}")
            nc.sync.dma_start(out=mkT, in_=mk[t * D : (t + 1) * D])
            val3 = vals.tile([P, K, D], bf16, tag="val3", name=f"val3_{i}")
            nc.scalar.dma_start(out=val3, in_=mv[rows])
            nk_t = nks.tile([P, D], bf16, tag="nk_t", name=f"nk_{i}")
            nc.gpsimd.dma_start(out=nk_t, in_=nk[rows])
            return {"mkT": mkT, "val3": val3, "nk_t": nk_t}

        def emit_scores(t, h):
            nk_t = h["nk_t"]
            junk = smalls.tile([P, D], bf16, tag="junk", name=f"junk_{t}")
            s1b = smalls.tile([P, 1], f32, tag="s1b", name=f"s1b_{t}")
            # s1 = a_b + nk.v1 — fused multiply + row-sum
            nc.vector.scalar_tensor_tensor(
                out=junk, in0=nk_t, scalar=1.0, in1=v1row,
                op0=mybir.AluOpType.bypass, op1=mybir.AluOpType.mult,
                accum_out=s1b,
            )
            nc.vector.tensor_add(out=s1b, in0=s1b, in1=ab_sb)
            # s2[n,k] = key[n,k].v2 on TensorE: stationary = transposed key
            # slice [d, n], moving = v2 [d, 1], PSUM column k
            ps = psums.tile([P, K], f32, tag="ps", name=f"ps_{t}")
            mkT3 = h["mkT"].rearrange("d (k p) -> d k p", p=P)
            for k in range(K):
                nc.tensor.matmul(
                    ps[:, k : k + 1], mkT3[:, k, :], v2col,
                    start=True, stop=True,
                )
            th = smalls.tile([P, K], bf16, tag="th", name=f"th_{t}")
            nc.scalar.activation(
                out=th, in_=ps, func=mybir.ActivationFunctionType.Tanh,
                bias=s1b, scale=1.0,
            )
            e_t = smalls.tile([P, K], f32, tag="e_t", name=f"e_{t}")
            sums = smalls.tile([P, 1], f32, tag="sums", name=f"sums_{t}")
            nc.scalar.activation(
                out=e_t, in_=th, func=mybir.ActivationFunctionType.Exp,
                accum_out=sums,
            )
            recip = smalls.tile([P, 1], f32, tag="recip", name=f"recip_{t}")
            nc.vector.reciprocal(out=recip, in_=sums)
            h["e_t"], h["recip"] = e_t, recip

        def emit_values(i, t, h):
            val3, e_t, recip = h["val3"], h["e_t"], h["recip"]
            rows = slice(t * P, (t + 1) * P)
            # out_t = sum_k val_k * e_k: bf16 multiply-accumulate sub-chains
            # (DVE 2x packed mode), recombined pairwise ending in fp32
            sub = []
            for c in range(NSUB):
                k0 = c * KSUB
                acc = accs.tile([P, D], bf16, tag=f"acc{c}", name=f"acc{c}_{i}")
                nc.vector.tensor_scalar_mul(
                    out=acc, in0=val3[:, k0, :], scalar1=e_t[:, k0 : k0 + 1]
                )
                for k in range(k0 + 1, k0 + KSUB):
                    nc.vector.scalar_tensor_tensor(
                        out=acc, in0=val3[:, k, :], scalar=e_t[:, k : k + 1],
                        in1=acc,
                        op0=mybir.AluOpType.mult, op1=mybir.AluOpType.add,
                    )
                sub.append(acc)
            while len(sub) > 1:
                nxt = []
                for c2 in range(0, len(sub) - 1, 2):
                    dst = accs.tile(
                        [P, D], mybir.dt.float32 if len(sub) == 2 else bf16,
                        tag=f"cmb{len(sub)}_{c2}", name=f"cmb{len(sub)}_{c2}_{i}",
                    )
                    nc.vector.tensor_add(out=dst, in0=sub[c2], in1=sub[c2 + 1])
                    nxt.append(dst)
                if len(sub) % 2:
                    nxt.append(sub[-1])
                sub = nxt
            out_t = outs.tile([P, D], bf16, tag="out_t", name=f"out_{i}")
            nc.scalar.activation(
                out=out_t, in_=sub[0],
                func=mybir.ActivationFunctionType.Copy, scale=recip,
            )
            nc.gpsimd.dma_start(out=out[rows], in_=out_t)

        handles = {}
        n_iters = repeat * n_tiles
        for i in range(n_iters + SKEW):
            if i < n_iters:
                h = emit_loads(i, i % n_tiles)
                emit_scores(i, h)
                handles[i] = h
            j = i - SKEW
            if j >= 0:
                emit_values(j, j % n_tiles, handles.pop(j))

    nc.finalize()
    return nc


_nc_cache = {}


def _get_nc():
    if "main" not in _nc_cache:
        _install_waitfix()
        _nc_cache["main"] = _build_kernel()
    return _nc_cache["main"]


def _host_prep(W1, W2, a_w, a_b):
    import ml_dtypes

    v1 = (W1.astype(np.float64).T @ a_w.astype(np.float64)).astype(np.float32)
    v2 = (W2.astype(np.float64).T @ a_w.astype(np.float64)).astype(np.float32)
    cb = np.zeros((128, 1 + D), np.float32)
    cb[:, 0] = v2
    cb[:, 1:] = v1[None, :]
    cf = np.full((128, 1), np.float32(a_b[0]), np.float32)
    return cb.astype(ml_dtypes.bfloat16), cf


def host_mkT(mk_pad, npc=NPC):
    """[NPAD,K,D] fp32 -> tile-major [(tile,d), (k,p)] bf16 slabs."""
    import ml_dtypes

    npad = mk_pad.shape[0]
    return np.ascontiguousarray(
        mk_pad.reshape(npad // P, P, K, D)
        .transpose(0, 3, 2, 1)
        .astype(ml_dtypes.bfloat16)
    ).reshape(-1, P * K)


def kernel(middle_key, nodes_key, middle_value, W1, W2, a_w, a_b):
    import ml_dtypes

    bf = ml_dtypes.bfloat16
    # middle_key: pad, then cast+transpose into tile-major [d, (k, p)] slabs
    mk_pad = np.zeros((NPAD, K, D), np.float32)
    mk_pad[:N] = np.ascontiguousarray(middle_key, np.float32)
    mkb = host_mkT(mk_pad)
    nkb = np.zeros((NPAD, D), bf)
    nkb[:N] = np.ascontiguousarray(nodes_key, np.float32).astype(bf)
    mvb = np.zeros((NPAD, K, D), bf)
    mvb[:N] = np.ascontiguousarray(middle_value, np.float32).astype(bf)
    cb, cf = _host_prep(W1, W2, a_w, a_b)

    nc = _get_nc()

    rpc = mkb.shape[0] // N_CORES
    in_maps = []
    for c in range(N_CORES):
        s = slice(c * NPC, (c + 1) * NPC)
        sk = slice(c * rpc, (c + 1) * rpc)
        in_maps.append(
            {"mk": mkb[sk], "nk": nkb[s], "mv": mvb[s], "cb": cb, "cf": cf}
        )

    from concourse import bass2jax

    results = bass2jax.run_bass_via_pjrt(nc, in_maps, n_cores=N_CORES)
    full = np.concatenate([r["out"] for r in results], axis=0)
    return full[:N].astype(np.float32)



# revision 18
# speedup vs baseline: 3.1704x; 3.1704x over previous
"""Trainium2 Bass kernel for nn_AttentionAggregate_Weight (gnn_message_passing).

Computes, per node n with K=32 neighbors and D=128 features:
    score[n,k] = tanh(nodes_key[n].v1 + middle_key[n,k].v2 + a_b)
    out[n,:]   = sum_k softmax_k(score)[n,k] * middle_value[n,k,:]
where v1 = W1.T @ a_w and v2 = W2.T @ a_w are folded on the host (the
reference's p1/p2 projections only ever appear dotted with a_w; tanh
outputs lie in [-1,1] so the softmax needs no max subtraction and the
denominator folds into one final per-node scale).

Distribution: pure data parallel over the node axis across 8 NeuronCores.
Nodes are host-padded 20000 -> 20480 so each core gets 2560 = 20 tiles of
exactly 128 nodes. middle_key streams as host-pretransposed fp8_e3m4
tile-major [d, (k, n)] slabs (halves the dominant-after-value traffic;
final rel-err ~8.7e-3 vs the 2e-2 gate); middle_value and nodes_key as
bf16 (value precision directly bounds output error, so mv stays 16-bit).

Engine split per 128-node tile (the baseline was DVE-bound, so work is
spread): TensorE computes the whole pre-tanh score tile in one PSUM
accumulation group — one matmul (stationary = host-transposed nk slab,
moving = v1 replicated K columns) adds the center-node term to all K
columns, then K tiny matmuls (stationary = contiguous mkT[:, k, :] fp8
slice, moving = v2) add each neighbor dot. ScalarE applies tanh (a_b via
the bias port) and exp (+fused row-sum). The weighted value sum runs as
bf16 multiply-accumulate sub-chains split across VectorE (2x-packed DVE
mode) and GpSimd, chains interleaved in emission so same-accumulator ops
are never back-to-back; VectorE merges the partials and ScalarE applies
the final 1/Z scale. Loads ride both HWDGE rings (middle_key+nodes_key
on SP, middle_value+out on ACT) leaving GpSimd free for MAC work. The
tile loop is software-pipelined two tiles deep; outputs return as bf16
and are upcast on the host.

Self-contained: hardcodes shapes/sharding; no file I/O.
"""

from contextlib import ExitStack

import numpy as np

N, K, D = 20000, 32, 128
N_CORES = 8
NPC = 2560  # padded nodes per core (20 tiles of 128)
NPAD = NPC * N_CORES
P = 128
SKEW = 2
BUFS = 3
MK_FP8 = True  # stage middle_key as fp8_e3m4 (scores only; ~7.5e-3 rel err)
KGP = 10  # value-chain k's on GpSimd (rest on VectorE)
NSUB_D = 4  # DVE sub-chains
NSUB_G = 2  # GpSimd sub-chains


# ---------------------------------------------------------------------------
# Wait legalization: this walrus build accepts at most ONE semaphore wait per
# instruction; split extras onto same-engine Drain carriers at the BIR level.
# ---------------------------------------------------------------------------
def _legalize_bir_waits(bir_bytes: bytes) -> bytes:
    import orjson

    m = orjson.loads(bir_bytes)
    n = 0
    for f in m.get("functions", []):
        for b in f.get("blocks", []):
            insts = b.get("instructions") or []
            out = []
            changed = False
            for ins in insts:
                si = ins.get("sync_info")
                waits = (si or {}).get("on_wait") or []
                if len(waits) > 1:
                    changed = True
                    for w in waits[:-1]:
                        n += 1
                        out.append(
                            {
                                "debug": ins.get("debug", 0),
                                "engine": ins.get("engine"),
                                "ins": [],
                                "outs": [],
                                "name": f"I-wfix-{n}",
                                "opcode": "Drain",
                                "sync_info": {"on_update": [], "on_wait": [w]},
                            }
                        )
                    si["on_wait"] = [waits[-1]]
                out.append(ins)
            if changed:
                b["instructions"] = out
    return orjson.dumps(m)


_waitfix_installed = False


def _install_waitfix():
    global _waitfix_installed
    if _waitfix_installed:
        return
    import concourse.bass as bass

    orig = bass.Bass.to_json_bytes

    def patched(self):
        return _legalize_bir_waits(orig(self))

    bass.Bass.to_json_bytes = patched
    _waitfix_installed = True


def _chains(ks, nsub):
    """Split k-indices into nsub round-robin-balanced contiguous chains."""
    if not ks:
        return []
    base, extra = divmod(len(ks), nsub)
    out = []
    pos = 0
    for c in range(nsub):
        ln = base + (1 if c < extra else 0)
        out.append(ks[pos : pos + ln])
        pos += ln
    return [c for c in out if c]


# ---------------------------------------------------------------------------
# Kernel builder (per-core: NPC nodes, P=128 per tile)
# ---------------------------------------------------------------------------
def _build_kernel(repeat=1, load_frac=1.0, dve_fd=None):
    import concourse.bass as bass
    import concourse.tile as tile
    from concourse import mybir

    f32 = mybir.dt.float32
    bf16 = mybir.dt.bfloat16
    mk_dt = mybir.dt.float8e3 if MK_FP8 else bf16
    n_tiles = NPC // P

    nc = bass.Bass()
    # host-pretransposed, tile-major: row block t*D..(t+1)*D is tile t's
    # [d, (k, p)] slab (k-major free so per-k stationary slices are contiguous)
    mk = nc.dram_tensor("mk", (n_tiles * D, P * K), mk_dt, kind="ExternalInput")
    # nodes_key, host-transposed per tile into [d, n] slabs (PE stationary)
    nk = nc.dram_tensor("nk", (n_tiles * D, P), bf16, kind="ExternalInput")
    mv = nc.dram_tensor("mv", (NPC, K, D), bf16, kind="ExternalInput")
    cb_d = nc.dram_tensor("cb", (128, 1 + K), bf16, kind="ExternalInput")
    cf_d = nc.dram_tensor("cf", (128, 1), f32, kind="ExternalInput")
    out = nc.dram_tensor("out", (NPC, D), bf16, kind="ExternalOutput")

    with tile.TileContext(nc) as tc, ExitStack() as ctx:
        singles = ctx.enter_context(tc.tile_pool(name="singles", bufs=1))
        keys = ctx.enter_context(tc.tile_pool(name="keys", bufs=BUFS + SKEW))
        vals = ctx.enter_context(tc.tile_pool(name="vals", bufs=BUFS + SKEW))
        nks = ctx.enter_context(tc.tile_pool(name="nks", bufs=BUFS))
        outs = ctx.enter_context(tc.tile_pool(name="outs", bufs=BUFS))
        smalls = ctx.enter_context(tc.tile_pool(name="smalls", bufs=BUFS + SKEW))
        accs = ctx.enter_context(tc.tile_pool(name="accs", bufs=2))
        psums = ctx.enter_context(tc.tile_pool(name="psums", bufs=2, space="PSUM"))

        cb = singles.tile([128, 1 + K], bf16)
        nc.gpsimd.dma_start(out=cb, in_=cb_d[:])
        cf = singles.tile([128, 1], f32)
        nc.gpsimd.dma_start(out=cf, in_=cf_d[:])
        v2col = cb[:, 0:1]
        v1rep = cb[:, 1 : 1 + K]  # v1 replicated K columns (moving operand)
        ab_sb = cf[0:P, 0:1]
        # dummy touches: engines observe the const-DMA semaphores up front
        dum = singles.tile([1, 2], f32)
        nc.vector.tensor_copy(out=dum[:, 0:1], in_=cf[0:1, 0:1])
        nc.scalar.activation(
            out=dum[:, 1:2], in_=cb[0:1, 0:1],
            func=mybir.ActivationFunctionType.Copy,
        )

        mk_cols = int(P * K * load_frac)  # <1.0: timing-probe variants only
        kv = max(1, int(K * load_frac))

        def emit_loads(i, t):
            rows = slice(t * P, (t + 1) * P)
            mkT = keys.tile([D, P * K], mk_dt, tag="mkT", name=f"mkT_{i}")
            nc.sync.dma_start(
                out=mkT[:, :mk_cols], in_=mk[t * D : (t + 1) * D, :mk_cols]
            )
            val3 = vals.tile([P, K, D], bf16, tag="val3", name=f"val3_{i}")
            nc.scalar.dma_start(out=val3[:, :kv, :], in_=mv[rows, :kv, :])
            nkT = nks.tile([D, P], bf16, tag="nkT", name=f"nk_{i}")
            nc.sync.dma_start(out=nkT, in_=nk[t * D : (t + 1) * D])
            return {"mkT": mkT, "val3": val3, "nkT": nkT}

        def emit_scores(i, h):
            # whole pre-tanh score tile in one PSUM accumulation group:
            # s1 broadcast via (nkT, v1rep), then K neighbor dots
            ps = psums.tile([P, K], f32, tag="ps", name=f"ps_{i}")
            nc.tensor.matmul(ps[:, 0:K], h["nkT"], v1rep, start=True, stop=False)
            mkT3 = h["mkT"].rearrange("d (k p) -> d k p", p=P)
            for k in range(K):
                nc.tensor.matmul(
                    ps[:, k : k + 1], mkT3[:, k, :], v2col,
                    start=False, stop=(k == K - 1), skip_group_check=True,
                )
            th = smalls.tile([P, K], bf16, tag="th", name=f"th_{i}")
            nc.scalar.activation(
                out=th, in_=ps, func=mybir.ActivationFunctionType.Tanh,
                bias=ab_sb, scale=1.0,
            )
            e_t = smalls.tile([P, K], f32, tag="e_t", name=f"e_{i}")
            sums = smalls.tile([P, 1], f32, tag="sums", name=f"sums_{i}")
            nc.scalar.activation(
                out=e_t, in_=th, func=mybir.ActivationFunctionType.Exp,
                accum_out=sums,
            )
            recip = smalls.tile([P, 1], f32, tag="recip", name=f"recip_{i}")
            nc.vector.reciprocal(out=recip, in_=sums)
            h["e_t"], h["recip"] = e_t, recip

        vfd = dve_fd or D  # <D: timing-probe variants only
        dve_ks = _chains(list(range(K - KGP)), NSUB_D)
        gp_ks = _chains(list(range(K - KGP, K)), NSUB_G)

        def emit_values(i, t, h):
            val3, e_t, recip = h["val3"], h["e_t"], h["recip"]
            rows = slice(t * P, (t + 1) * P)
            # out_t = sum_k val_k * e_k: bf16 multiply-accumulate sub-chains
            # split DVE/GpSimd, interleaved so same-acc ops never adjoin
            chains = [(nc.vector, f"d{c}", ks, vfd) for c, ks in enumerate(dve_ks)]
            chains += [(nc.gpsimd, f"g{c}", ks, D) for c, ks in enumerate(gp_ks)]
            acc_of = {}
            for tag_i, (eng, cn, ks, fd) in enumerate(chains):
                acc_of[cn] = accs.tile(
                    [P, D], bf16, tag=f"acc_{cn}", name=f"acc{cn}_{i}"
                )
            depth = max(len(ks) for _, _, ks, _ in chains)
            for step in range(depth):
                for eng, cn, ks, fd in chains:
                    if step >= len(ks):
                        continue
                    k = ks[step]
                    acc = acc_of[cn]
                    if step == 0:
                        eng.tensor_scalar_mul(
                            out=acc[:, :fd], in0=val3[:, k, :fd],
                            scalar1=e_t[:, k : k + 1],
                        )
                    else:
                        eng.scalar_tensor_tensor(
                            out=acc[:, :fd], in0=val3[:, k, :fd],
                            scalar=e_t[:, k : k + 1], in1=acc[:, :fd],
                            op0=mybir.AluOpType.mult, op1=mybir.AluOpType.add,
                        )
            # merge: GpSimd folds its own chains, DVE folds the rest + final
            gp_accs = [acc_of[f"g{c}"] for c in range(len(gp_ks))]
            while len(gp_accs) > 1:
                nxt = []
                for c2 in range(0, len(gp_accs) - 1, 2):
                    dst = accs.tile(
                        [P, D], bf16, tag=f"gcmb{len(gp_accs)}_{c2}",
                        name=f"gcmb{len(gp_accs)}_{c2}_{i}",
                    )
                    nc.gpsimd.tensor_add(
                        out=dst, in0=gp_accs[c2], in1=gp_accs[c2 + 1]
                    )
                    nxt.append(dst)
                if len(gp_accs) % 2:
                    nxt.append(gp_accs[-1])
                gp_accs = nxt
            sub = [acc_of[f"d{c}"] for c in range(len(dve_ks))] + gp_accs
            while len(sub) > 1:
                nxt = []
                for c2 in range(0, len(sub) - 1, 2):
                    dst = accs.tile(
                        [P, D], bf16, tag=f"cmb{len(sub)}_{c2}",
                        name=f"cmb{len(sub)}_{c2}_{i}",
                    )
                    nc.vector.tensor_add(out=dst, in0=sub[c2], in1=sub[c2 + 1])
                    nxt.append(dst)
                if len(sub) % 2:
                    nxt.append(sub[-1])
                sub = nxt
            out_t = outs.tile([P, D], bf16, tag="out_t", name=f"out_{i}")
            nc.scalar.activation(
                out=out_t, in_=sub[0],
                func=mybir.ActivationFunctionType.Copy, scale=recip,
            )
            nc.scalar.dma_start(out=out[rows], in_=out_t)

        handles = {}
        n_iters = repeat * n_tiles
        for i in range(n_iters + SKEW):
            if i < n_iters:
                h = emit_loads(i, i % n_tiles)
                emit_scores(i, h)
                handles[i] = h
            j = i - SKEW
            if j >= 0:
                emit_values(j, j % n_tiles, handles.pop(j))

    nc.finalize()
    return nc


_nc_cache = {}


def _get_nc():
    if "main" not in _nc_cache:
        _install_waitfix()
        _nc_cache["main"] = _build_kernel()
    return _nc_cache["main"]


def _host_prep(W1, W2, a_w, a_b):
    import ml_dtypes

    v1 = (W1.astype(np.float64).T @ a_w.astype(np.float64)).astype(np.float32)
    v2 = (W2.astype(np.float64).T @ a_w.astype(np.float64)).astype(np.float32)
    cb = np.zeros((128, 1 + K), np.float32)
    cb[:, 0] = v2
    cb[:, 1:] = v1[:, None]
    cf = np.full((128, 1), np.float32(a_b[0]), np.float32)
    return cb.astype(ml_dtypes.bfloat16), cf


def host_mkT(mk_pad, npc=NPC):
    """[NPAD,K,D] fp32 -> tile-major [(tile,d), (k,p)] slabs (fp8/bf16)."""
    import ml_dtypes

    dt = ml_dtypes.float8_e3m4 if MK_FP8 else ml_dtypes.bfloat16
    npad = mk_pad.shape[0]
    return np.ascontiguousarray(
        mk_pad.reshape(npad // P, P, K, D).transpose(0, 3, 2, 1).astype(dt)
    ).reshape(-1, P * K)


def host_nkT(nk_pad):
    """[NPAD,D] -> tile-major [(tile,d), n] bf16 slabs (PE stationary)."""
    import ml_dtypes

    npad = nk_pad.shape[0]
    return np.ascontiguousarray(
        nk_pad.reshape(npad // P, P, D)
        .transpose(0, 2, 1)
        .astype(ml_dtypes.bfloat16)
    ).reshape(-1, P)


def kernel(middle_key, nodes_key, middle_value, W1, W2, a_w, a_b):
    import ml_dtypes

    bf = ml_dtypes.bfloat16
    # middle_key: pad, then cast+transpose into tile-major [d, (k, p)] slabs
    mk_pad = np.zeros((NPAD, K, D), np.float32)
    mk_pad[:N] = np.ascontiguousarray(middle_key, np.float32)
    mkb = host_mkT(mk_pad)
    nk_pad = np.zeros((NPAD, D), np.float32)
    nk_pad[:N] = np.ascontiguousarray(nodes_key, np.float32)
    nkb = host_nkT(nk_pad)
    mvb = np.zeros((NPAD, K, D), bf)
    mvb[:N] = np.ascontiguousarray(middle_value, np.float32).astype(bf)
    cb, cf = _host_prep(W1, W2, a_w, a_b)

    nc = _get_nc()

    rpc = mkb.shape[0] // N_CORES
    in_maps = []
    for c in range(N_CORES):
        s = slice(c * NPC, (c + 1) * NPC)
        sk = slice(c * rpc, (c + 1) * rpc)
        in_maps.append(
            {"mk": mkb[sk], "nk": nkb[sk], "mv": mvb[s], "cb": cb, "cf": cf}
        )

    from concourse import bass2jax

    results = bass2jax.run_bass_via_pjrt(nc, in_maps, n_cores=N_CORES)
    full = np.concatenate([r["out"] for r in results], axis=0)
    return full[:N].astype(np.float32)
